# revision 2
# baseline (speedup 1.0000x reference)
"""DiagPooling (segment-reduce over square-image diagonals) on 8 NeuronCores.

Input  x: [8, 128, 512, 512] f32. Output: [8, 1, 513] f32 — per batch, the
mean over (channels, diagonal) of each diagonal offset in [-256, 256].

Sharding: batch b -> core b (data parallel, no communication).

Per-core pipeline:
1. View the padded per-channel image (262144 + 512 elements) as the stride-513
   matrix P[q, r] = flat[513*q + r]; every diagonal of the image is a COLUMN
   of P (column r holds diagonal o = r for rows q <= 511 - r and o = r - 513
   for q >= 512 - r). Assign rows q = 4p + j (j in [0,4)) to partition p, so
   each partition reads 4*513*4 = 8208 CONTIGUOUS bytes per channel — the
   stream lands directly in diagonal layout at full HBM descriptor efficiency,
   with no re-layout pass at all. Channels are streamed 4 per DMA (4 MiB).
2. Accumulate the channel sum in this layout, alternating tiles between the
   Vector and GpSimd engines so neither engine's add chain ever throttles the
   DMA stream (a single DVE chain caps at ~330 GB/s; the stream runs ~420).
3. The wanted diagonals o in [-256, 256] are selected by a precomputed mask
   folded with 1/(C*diag_len): V = (acc_v + acc_g) * w, fold j, then a
   ones-vector matmul folds partitions. The 512-element overrun of each
   channel's last partition lands in cells the mask zeroes (q = 511, r >= 1),
   so the padding needs no special handling.
"""

import numpy as np

import concourse.bass as bass
import concourse.bacc as bacc
import concourse.mybir as mybir
from concourse import tile
from concourse.bass_utils import run_bass_kernel_spmd

B, C, H = 8, 128, 512
R = H + 1               # 513: columns of the strided view
NJ = 4                  # rows per partition: row q = 4*p + j
F = NJ * R              # 2052: per-channel P-layout width per partition
CH_ELEMS = H * H        # 262144 elements per (b, c) image
KC = 4                  # channels per stream DMA (4 MiB transfers)
PAD = H                 # flat pad so the last partition's read stays in bounds
N_IN = C * CH_ELEMS
F32 = mybir.dt.float32


def _mask_qr() -> np.ndarray:
    """[512, 513] f64: wanted(q, r) / (C * diag_len)."""
    q = np.arange(H, dtype=np.int64)[:, None]
    r = np.arange(R, dtype=np.int64)[None, :]
    prefix = (r <= H // 2) & (q + r <= H - 1)            # diagonal o = r
    suffix = (r > H // 2) & (q + r >= H) & (q <= H - 2)  # o = r - 513
    mask = prefix | suffix
    o = np.where(r <= H // 2, r, r - R)
    denom = float(C) * (H - np.abs(o)).astype(np.float64)
    return mask.astype(np.float64) / denom


def _build_weights() -> np.ndarray:
    """[128, F] f32: the mask in the SBUF layout (row q = 4p + j ->
    partition p, free column j*513 + r)."""
    return _mask_qr().reshape(128, F).astype(np.float32)


def _build_program():
    nc = bacc.Bacc("TRN2", target_bir_lowering=False, debug=False, num_devices=B)
    xp = nc.dram_tensor("x", [N_IN + PAD], F32, kind="ExternalInput")
    wt = nc.dram_tensor("w", [128, F], F32, kind="ExternalInput")
    out_t = nc.dram_tensor("out", [1, R], F32, kind="ExternalOutput")

    NBUFS = 4

    with tile.TileContext(nc) as tc:
        with (
            tc.tile_pool(name="consts", bufs=1) as consts,
            tc.tile_pool(name="accp", bufs=1) as accp,
            tc.tile_pool(name="loadp", bufs=NBUFS) as loadp,
            tc.tile_pool(name="outp", bufs=1) as outp,
            tc.tile_pool(name="psum", bufs=2, space=bass.MemorySpace.PSUM) as psump,
        ):
            # epilogue constants load on the otherwise-idle scalar ring so the
            # sync-ring channel stream is never delayed
            w_tile = consts.tile([128, F], F32)
            nc.scalar.dma_start(out=w_tile[:], in_=wt.ap())
            ones = consts.tile([128, 1], F32)
            nc.gpsimd.memset(ones[:], 1.0)

            acc_v = accp.tile([128, F], F32)
            acc_g = accp.tile([128, F], F32)

            # channel plan: 31 four-channel tiles, then 2+2 so both engines
            # finish their last (small) adds in parallel right as the stream
            # ends, shortening the serial tail
            plan = []
            c0, k = 0, 0
            while c0 < C - KC:
                plan.append((c0, KC, k % 2))
                c0 += KC
                k += 1
            plan.append((c0, 2, 0))
            plan.append((c0 + 2, 2, 1))

            started = [False, False]
            for c0, ncs, ei in plan:
                t = loadp.tile([128, ncs * F], F32)
                nc.sync.dma_start(
                    out=t[:],
                    in_=bass.AP(xp, c0 * CH_ELEMS, [[F, 128], [CH_ELEMS, ncs], [1, F]]),
                )
                eng = nc.vector if ei == 0 else nc.gpsimd
                acc = acc_v if ei == 0 else acc_g
                for i in range(ncs):
                    sl = t[:, i * F : (i + 1) * F]
                    if not started[ei]:
                        eng.tensor_copy(out=acc[:], in_=sl)
                        started[ei] = True
                    else:
                        eng.tensor_add(out=acc[:], in0=acc[:], in1=sl)

            # masked fold: V = (acc_v + acc_g) * w, split across DVE/GpSimd;
            # fold the 4 j-rows; ones-matmul folds the 128 partitions
            hf = 2 * R  # 1026
            va = outp.tile([128, hf], F32)
            vb = outp.tile([128, hf], F32)
            nc.vector.tensor_add(out=va[:], in0=acc_v[:, 0:hf], in1=acc_g[:, 0:hf])
            nc.gpsimd.tensor_add(
                out=vb[:], in0=acc_v[:, hf : 2 * hf], in1=acc_g[:, hf : 2 * hf]
            )
            nc.vector.tensor_mul(out=va[:], in0=va[:], in1=w_tile[:, 0:hf])
            nc.gpsimd.tensor_mul(out=vb[:], in0=vb[:], in1=w_tile[:, hf : 2 * hf])
            ua = outp.tile([128, R], F32)
            ub = outp.tile([128, R], F32)
            nc.vector.tensor_add(out=ua[:], in0=va[:, 0:R], in1=va[:, R : 2 * R])
            nc.gpsimd.tensor_add(out=ub[:], in0=vb[:, 0:R], in1=vb[:, R : 2 * R])
            u = outp.tile([128, R], F32)
            nc.vector.tensor_add(out=u[:], in0=ua[:], in1=ub[:])

            ps_a = psump.tile([1, 512], F32)
            ps_b = psump.tile([1, 1], F32)
            nc.tensor.matmul(ps_a[:], ones[:], u[:, 0:512], start=True, stop=True)
            nc.tensor.matmul(ps_b[:], ones[:], u[:, 512:513], start=True, stop=True)
            res = outp.tile([1, R], F32)
            nc.vector.tensor_copy(out=res[:, 0:512], in_=ps_a[:])
            nc.vector.tensor_copy(out=res[:, 512:513], in_=ps_b[:])
            nc.sync.dma_start(out=out_t.ap(), in_=res[:])

    nc.compile()
    return nc


_CACHE = {}


def kernel(x, _trace=False, _trace_cores=None) -> np.ndarray:
    x = np.asarray(x, dtype=np.float32)
    assert x.shape == (B, C, H, H), x.shape

    if "nc" not in _CACHE:
        _CACHE["nc"] = _build_program()
        _CACHE["w"] = _build_weights()
    nc = _CACHE["nc"]
    w = _CACHE["w"]

    in_maps = []
    for b in range(B):
        xb = np.empty(N_IN + PAD, dtype=np.float32)
        xb[:N_IN] = np.asarray(x[b]).reshape(-1)
        xb[N_IN:] = 0.0
        in_maps.append({"x": xb, "w": w})
    result = run_bass_kernel_spmd(
        nc,
        in_maps,
        core_ids=list(range(B)),
        trace=_trace,
        trace_cores=_trace_cores,
    )
    _CACHE["last_result"] = result

    out = np.empty((B, 1, R), dtype=np.float32)
    for b in range(B):
        r = result.results[b]["out"].reshape(R)
        # column r -> offset o = r (r <= 256) / r - 513 (r >= 257);
        # output index n = o + 256
        out[b, 0, :] = np.concatenate([r[R - 256 :], r[: R - 256]])
    return out


# revision 6
# speedup vs baseline: 1.0311x; 1.0311x over previous
"""DiagPooling (segment-reduce over square-image diagonals) on 8 NeuronCores.

Input  x: [8, 128, 512, 512] f32. Output: [8, 1, 513] f32 — per batch, the
mean over (channels, diagonal) of each diagonal offset in [-256, 256].

Sharding: batch b -> core b (data parallel, no communication).

Per-core pipeline:
1. View the padded per-channel image (262144 + 512 elements) as the stride-513
   matrix P[q, r] = flat[513*q + r]; every diagonal of the image is a COLUMN
   of P (column r holds diagonal o = r for rows q <= 511 - r and o = r - 513
   for q >= 512 - r). Assign rows q = 4p + j (j in [0,4)) to partition p, so
   each partition reads 4*513*4 = 8208 CONTIGUOUS bytes per channel — the
   stream lands directly in diagonal layout at full HBM descriptor efficiency,
   with no re-layout pass at all. Channels stream 8 per DMA (8 MiB).
2. Accumulate with chained DVE tensor_adds, 6 channels per DMA: the adds run
   back-to-back (~35 ns gap); only the per-tile DMA semaphore wait (~1 us)
   interrupts the chain, so the DVE chain rate (~425 GB/s) matches the ~420
   GB/s DMA stream instead of throttling it to ~330 GB/s as with per-channel
   DMAs. (Splitting adds across DVE+GpSimd does NOT work: concurrent
   elementwise ops on the two engines contend for SBUF ports and both drop to
   ~38% speed. tensor_reduce over a strided channel axis is no faster — DVE
   fp32 is ~1 elem/cycle either way — and wedged the device on hardware.)
3. The wanted diagonals o in [-256, 256] are selected by a precomputed mask
   folded with 1/(C*diag_len): V = acc * w, tensor_reduce over the 4 j-rows,
   then a ones-vector matmul folds partitions. The 512-element overrun of each
   channel's last partition lands in cells the mask zeroes (q = 511, r >= 1),
   so the padding needs no special handling.
"""

import numpy as np

import concourse.bass as bass
import concourse.bacc as bacc
import concourse.mybir as mybir
from concourse import tile
from concourse.bass_utils import run_bass_kernel_spmd

B, C, H = 8, 128, 512
R = H + 1               # 513: columns of the strided view
NJ = 4                  # rows per partition: row q = 4*p + j
F = NJ * R              # 2052: per-channel P-layout width per partition
CH_ELEMS = H * H        # 262144 elements per (b, c) image
PAD = H                 # flat pad so the last partition's read stays in bounds
N_IN = C * CH_ELEMS
F32 = mybir.dt.float32

# channel plan: big tiles for stream throughput, small ones at the end so the
# post-stream serial tail (last tile's adds) is short
KC_BIG, N_BIG = 6, 21   # 21 x 6-channel tiles (126 channels)
TAIL = [1, 1]           # + 2 x 1-channel tiles


def _mask_qr() -> np.ndarray:
    """[512, 513] f64: wanted(q, r) / (C * diag_len)."""
    q = np.arange(H, dtype=np.int64)[:, None]
    r = np.arange(R, dtype=np.int64)[None, :]
    prefix = (r <= H // 2) & (q + r <= H - 1)            # diagonal o = r
    suffix = (r > H // 2) & (q + r >= H) & (q <= H - 2)  # o = r - 513
    mask = prefix | suffix
    o = np.where(r <= H // 2, r, r - R)
    denom = float(C) * (H - np.abs(o)).astype(np.float64)
    return mask.astype(np.float64) / denom


def _build_weights() -> np.ndarray:
    """[128, F] f32: the mask in the SBUF layout (row q = 4p + j ->
    partition p, free column j*513 + r)."""
    return _mask_qr().reshape(128, F).astype(np.float32)


def _build_program():
    nc = bacc.Bacc("TRN2", target_bir_lowering=False, debug=False, num_devices=B)
    xp = nc.dram_tensor("x", [N_IN + PAD], F32, kind="ExternalInput")
    wt = nc.dram_tensor("w", [128, F], F32, kind="ExternalInput")
    out_t = nc.dram_tensor("out", [1, R], F32, kind="ExternalOutput")

    with tile.TileContext(nc) as tc:
        with (
            tc.tile_pool(name="consts", bufs=1) as consts,
            tc.tile_pool(name="accp", bufs=1) as accp,
            tc.tile_pool(name="loadp", bufs=3) as loadp,
            tc.tile_pool(name="outp", bufs=1) as outp,
            tc.tile_pool(name="psum", bufs=2, space=bass.MemorySpace.PSUM) as psump,
        ):
            # epilogue constants load on the otherwise-idle scalar ring so the
            # sync-ring channel stream is never delayed
            w_tile = consts.tile([128, F], F32)
            nc.scalar.dma_start(out=w_tile[:], in_=wt.ap())
            ones = consts.tile([128, 1], F32)
            nc.gpsimd.memset(ones[:], 1.0)

            acc = accp.tile([128, F], F32)

            plan = [(k * KC_BIG, KC_BIG) for k in range(N_BIG)]
            c0 = N_BIG * KC_BIG
            for ncs in TAIL:
                plan.append((c0, ncs))
                c0 += ncs
            assert c0 == C

            first = True
            for c0, ncs in plan:
                t = loadp.tile([128, KC_BIG * F], F32)
                nc.sync.dma_start(
                    out=t[:, : ncs * F],
                    in_=bass.AP(xp, c0 * CH_ELEMS, [[F, 128], [CH_ELEMS, ncs], [1, F]]),
                )
                for i in range(ncs):
                    sl = t[:, i * F : (i + 1) * F]
                    if first:
                        nc.vector.tensor_copy(out=acc[:], in_=sl)
                        first = False
                    else:
                        nc.vector.tensor_add(out=acc[:], in0=acc[:], in1=sl)

            # masked fold: V = acc * w; u = sum_j V_j; means = ones^T @ u
            v = outp.tile([128, F], F32)
            nc.vector.tensor_mul(out=v[:], in0=acc[:], in1=w_tile[:])
            u = outp.tile([128, R], F32)
            nc.vector.tensor_add(out=u[:], in0=v[:, 0:R], in1=v[:, R : 2 * R])
            nc.vector.tensor_add(out=u[:], in0=u[:], in1=v[:, 2 * R : 3 * R])
            nc.vector.tensor_add(out=u[:], in0=u[:], in1=v[:, 3 * R : 4 * R])

            ps_a = psump.tile([1, 512], F32)
            ps_b = psump.tile([1, 1], F32)
            nc.tensor.matmul(ps_a[:], ones[:], u[:, 0:512], start=True, stop=True)
            nc.tensor.matmul(ps_b[:], ones[:], u[:, 512:513], start=True, stop=True)
            res = outp.tile([1, R], F32)
            nc.vector.tensor_copy(out=res[:, 0:512], in_=ps_a[:])
            nc.vector.tensor_copy(out=res[:, 512:513], in_=ps_b[:])
            nc.sync.dma_start(out=out_t.ap(), in_=res[:])

    nc.compile()
    return nc


_CACHE = {}


def kernel(x, _trace=False, _trace_cores=None) -> np.ndarray:
    x = np.asarray(x, dtype=np.float32)
    assert x.shape == (B, C, H, H), x.shape

    if "nc" not in _CACHE:
        _CACHE["nc"] = _build_program()
        _CACHE["w"] = _build_weights()
    nc = _CACHE["nc"]
    w = _CACHE["w"]

    in_maps = []
    for b in range(B):
        xb = np.empty(N_IN + PAD, dtype=np.float32)
        xb[:N_IN] = np.asarray(x[b]).reshape(-1)
        xb[N_IN:] = 0.0
        in_maps.append({"x": xb, "w": w})
    result = run_bass_kernel_spmd(
        nc,
        in_maps,
        core_ids=list(range(B)),
        trace=_trace,
        trace_cores=_trace_cores,
    )
    _CACHE["last_result"] = result

    out = np.empty((B, 1, R), dtype=np.float32)
    for b in range(B):
        r = result.results[b]["out"].reshape(R)
        # column r -> offset o = r (r <= 256) / r - 513 (r >= 257);
        # output index n = o + 256
        out[b, 0, :] = np.concatenate([r[R - 256 :], r[: R - 256]])
    return out
